# revision 4
# baseline (speedup 1.0000x reference)
"""Trainium2 Bass kernel for nn_CorrLoss: margin-ranking loss over a Gram matrix.

loss = mean_i relu( max_{j: t_j != t_i} corr[i,j] - min_{j: t_j == t_i} corr[i,j] + 40 )
with corr = feat @ feat.T, feat [4096, 512] f32, targets [4096] int.

Strategy (row-data-parallel over 8 NeuronCores, class-sorted layout):
- Host sorts rows by class. Core c owns sorted rows [512c, 512c+512); its
  column order is the sorted order rotated by -512c, so the core's own rows
  are exactly columns [0, 512) and the stationary matmul operand slices
  directly out of the feature tile (no separate local-feature DMA).
- The same-class mask is folded into the matmul: the contraction dim is
  extended by a one-hot class block scaled by -BIG on the stationary side,
  so PSUM holds scr = corr - BIG*same. Then an = rowmax(scr) (positives are
  pushed BIG below any negative) and ap = rowmin(scr over the positive
  window) + BIG (offset positives always win the min; unmasked negatives in
  the window can't poison it).
- Class-sorted columns put each core's positives in cols [0, head_w) plus a
  small wrapped tail at the end, so the mask matmul and min-reduce only
  touch chunks {0, 1, 7} instead of all 8.
- Features/one-hots are bf16 (full-rate PE, half DMA); PSUM and all
  reductions stay f32.
- DMA: 4 column-quarter transfers (each carries all 4 k-slices, 2KB
  descriptors) spread across the SP/Act/Pool DGE queues with per-quarter
  semaphores, so issue overhead doesn't serialize and the PE starts after
  one quarter.
- Scalar engine drains each PSUM chunk to SBUF (bank frees at copy speed,
  keeping the PE stall-free at full clock); DVE runs wide half-block
  reduces from SBUF.
"""
import sys
from contextlib import ExitStack

import numpy as np

sys.path.insert(0, "/opt/trn_rl_repo")

import concourse.bass as bass  # noqa: E402
from concourse import mybir  # noqa: E402
from concourse.bass_utils import run_bass_kernel_spmd  # noqa: E402

import ml_dtypes  # noqa: E402

BF16 = ml_dtypes.bfloat16

N_CORES = 8
N = 4096                # total rows
D = 512                 # feature dim
M = N // N_CORES        # 512 local rows per core
KT = D // 128           # 4 feature k-chunks
MT = M // 128           # 4 row blocks of 128
NCHUNK = 512            # psum chunk width
NT = N // NCHUNK        # 8 col chunks
NQ = 4                  # fT DMA column quarters
QW = N // NQ            # 1024 cols per quarter
MARGIN = 40.0
BIG = 2048.0

# positive-window geometry (multiples of chunk/tile sizes, validated on host)
KOH = 16                # max distinct classes per core (one-hot depth)
HEADC = 2               # head window = chunks [0, HEADC) -> cols [0, 1024)
TAILW = 256             # tail window = last TAILW cols of chunk NT-1
WCOLS = HEADC * NCHUNK + TAILW

_CACHE = {}


def _build():
    f32 = mybir.dt.float32
    bf = mybir.dt.bfloat16
    op = mybir.AluOpType
    nc = bass.Bass("TRN2", target_bir_lowering=False, debug=False)

    # fTr[q, k*128+p, c] = feature (k*128+p) of column (q*QW + c)
    fTr = nc.declare_dram_parameter("fTr", [NQ, D, QW], bf, isOutput=False)
    # oh[:, 0:512] = -BIG * onehot(local rows); oh[:, 512:] = onehot(window cols)
    oh = nc.declare_dram_parameter("oh", [KOH, M + WCOLS], bf, isOutput=False)
    # out2[:, 0:4] = row min over window (ap - BIG); out2[:, 4:8] = row max (an)
    out2 = nc.declare_dram_parameter("out2", [128, 2 * MT], f32, isOutput=True)

    with ExitStack() as ctx:
        fTs = ctx.enter_context(nc.sbuf_tensor("fTs", [128, KT, N], bf))
        ohs = ctx.enter_context(nc.sbuf_tensor("ohs", [128, M + WCOLS], bf))
        scr = ctx.enter_context(nc.sbuf_tensor("scr", [128, MT, N], f32))
        an_acc = ctx.enter_context(nc.sbuf_tensor("an_acc", [128, MT, 2], f32))
        ap_acc = ctx.enter_context(nc.sbuf_tensor("ap_acc", [128, MT, 2], f32))
        out_sb = ctx.enter_context(nc.sbuf_tensor("out_sb", [128, 2 * MT], f32))
        pt = [ctx.enter_context(nc.psum_tensor(f"pt{i}", [128, NCHUNK], f32))
              for i in range(8)]
        oh_sem = ctx.enter_context(nc.semaphore("oh_sem"))
        q_sem = [ctx.enter_context(nc.semaphore(f"q_sem{q}")) for q in range(NQ)]
        mm_sem = ctx.enter_context(nc.semaphore("mm_sem"))
        cp_sem = ctx.enter_context(nc.semaphore("cp_sem"))
        done_sem = ctx.enter_context(nc.semaphore("done_sem"))
        out_sem = ctx.enter_context(nc.semaphore("out_sem"))
        block = ctx.enter_context(nc.Block())

        def ft_piece(sync_or_eng, q):
            return sync_or_eng.dma_start(
                fTs[:, :, q * QW:(q + 1) * QW],
                fTr[q].rearrange("(k p) c -> p k c", k=KT),
            )

        # DMA distribution: SP carries q0+q3, Act q1, Pool oh+q2.
        @block.sync
        def _(sync):
            ft_piece(sync, 0).then_inc(q_sem[0], 16)
            ft_piece(sync, 3).then_inc(q_sem[3], 16)
            sync.wait_ge(done_sem, 1)
            sync.dma_start(out2[:], out_sb[:]).then_inc(out_sem, 16)
            sync.wait_ge(out_sem, 16)

        @block.gpsimd
        def _(gpsimd):
            gpsimd.dma_start(ohs[0:KOH, :], oh[:, :]).then_inc(oh_sem, 16)
            ft_piece(gpsimd, 2).then_inc(q_sem[2], 16)

        @block.tensor
        def _(tensor):
            for n in range(NT):
                q = (n * NCHUNK) // QW
                tensor.wait_ge(q_sem[q], 16)
                if n == 0:
                    tensor.wait_ge(oh_sem, 16)
                for m in range(MT):
                    c = n * MT + m
                    b = c % 8
                    if c >= 8:
                        tensor.wait_ge(cp_sem, c - 7)
                    has_mask = (n < HEADC) or (n == NT - 1)
                    for k in range(KT):
                        mm = nc.tensor.matmul(
                            pt[b][:],
                            fTs[:, k, m * 128:(m + 1) * 128],
                            fTs[:, k, n * NCHUNK:(n + 1) * NCHUNK],
                            start=(k == 0),
                            stop=(k == KT - 1 and not has_mask))
                        if k == KT - 1 and not has_mask:
                            mm.then_inc(mm_sem, 1)
                    if n < HEADC:
                        mm = nc.tensor.matmul(
                            pt[b][:],
                            ohs[0:KOH, m * 128:(m + 1) * 128],
                            ohs[0:KOH, M + n * NCHUNK:M + (n + 1) * NCHUNK],
                            start=False, stop=True)
                        mm.then_inc(mm_sem, 1)
                    elif n == NT - 1:
                        mm = nc.tensor.matmul(
                            pt[b][:, NCHUNK - TAILW:NCHUNK],
                            ohs[0:KOH, m * 128:(m + 1) * 128],
                            ohs[0:KOH, M + HEADC * NCHUNK:M + WCOLS],
                            start=False, stop=True, skip_group_check=True)
                        mm.then_inc(mm_sem, 1)

        @block.scalar
        def _(scalar):
            ft_piece(scalar, 1).then_inc(q_sem[1], 16)
            for n in range(NT):
                for m in range(MT):
                    c = n * MT + m
                    b = c % 8
                    scalar.wait_ge(mm_sem, c + 1)
                    cp = nc.scalar.copy(
                        scr[:, m, n * NCHUNK:(n + 1) * NCHUNK], pt[b][:])
                    cp.then_inc(cp_sem, 1)

        @block.vector
        def _(vector):
            H = NT // 2
            for m in range(MT):
                # first half of block m copied once cp count reaches 3*MT+m+1
                vector.wait_ge(cp_sem, (H - 1) * MT + m + 1)
                nc.vector.tensor_reduce(
                    an_acc[:, m, 0:1], scr[:, m, 0:H * NCHUNK],
                    axis=mybir.AxisListType.X, op=op.max)
                nc.vector.tensor_reduce(
                    ap_acc[:, m, 0:1], scr[:, m, 0:HEADC * NCHUNK],
                    axis=mybir.AxisListType.X, op=op.min)
            for m in range(MT):
                vector.wait_ge(cp_sem, (NT - 1) * MT + m + 1)
                nc.vector.tensor_reduce(
                    an_acc[:, m, 1:2], scr[:, m, H * NCHUNK:N],
                    axis=mybir.AxisListType.X, op=op.max)
                nc.vector.tensor_reduce(
                    ap_acc[:, m, 1:2], scr[:, m, N - TAILW:N],
                    axis=mybir.AxisListType.X, op=op.min)
                nc.vector.tensor_reduce(
                    out_sb[:, MT + m:MT + m + 1], an_acc[:, m, :],
                    axis=mybir.AxisListType.X, op=op.max)
                fin = nc.vector.tensor_reduce(
                    out_sb[:, m:m + 1], ap_acc[:, m, :],
                    axis=mybir.AxisListType.X, op=op.min)
                if m == MT - 1:
                    fin.then_inc(done_sem, 1)
    return nc


def _prep_inputs(feat: np.ndarray, targets: np.ndarray):
    """Sort rows by class and build per-core rotated inputs."""
    feat = np.asarray(feat, dtype=np.float32)
    tg = np.asarray(targets).astype(np.int64).ravel()

    order = np.argsort(tg, kind="stable")
    ts = tg[order]                       # sorted targets
    fT_s = np.ascontiguousarray(feat[order].T)   # [512, 4096] f32, sorted cols

    in_maps = []
    for c in range(N_CORES):
        base = M * c
        tcol = np.roll(ts, -base)
        fTp = np.roll(fT_s, -base, axis=1).astype(BF16)
        fTr = np.ascontiguousarray(
            fTp.reshape(D, NQ, QW).transpose(1, 0, 2))   # [NQ, D, QW]

        c0 = int(ts[base])
        c1 = int(ts[base + M - 1])
        span = c1 - c0 + 1
        p0 = int(np.searchsorted(ts, c0, "left"))
        p1 = int(np.searchsorted(ts, c1, "right"))
        head_w = p1 - base
        tail_w = base - p0
        assert span <= KOH, f"class span {span} > {KOH}"
        assert head_w <= HEADC * NCHUNK, f"head window {head_w}"
        assert tail_w <= TAILW, f"tail window {tail_w}"

        ohx = np.zeros((KOH, M + WCOLS), dtype=np.float32)
        tloc = tcol[:M]
        ohx[tloc - c0, np.arange(M)] = -BIG
        hidx = tcol[:HEADC * NCHUNK] - c0
        hsel = (hidx >= 0) & (hidx < span)
        ohx[hidx[hsel], M + np.nonzero(hsel)[0]] = 1.0
        tidx = tcol[N - TAILW:] - c0
        tsel = (tidx >= 0) & (tidx < span)
        ohx[tidx[tsel], M + HEADC * NCHUNK + np.nonzero(tsel)[0]] = 1.0

        in_maps.append({"fTr": fTr, "oh": ohx.astype(BF16)})
    return in_maps


def kernel(feat: np.ndarray, targets: np.ndarray) -> np.ndarray:
    in_maps = _prep_inputs(feat, targets)

    if "nc" not in _CACHE:
        _CACHE["nc"] = _build()
    nc = _CACHE["nc"]

    res = run_bass_kernel_spmd(nc, in_maps, list(range(N_CORES)))
    total = 0.0
    for c in range(N_CORES):
        o = res.results[c]["out2"].astype(np.float64)
        ap = o[:, :MT] + BIG
        an = o[:, MT:]
        total += np.maximum(an - ap + MARGIN, 0.0).sum()
    return np.asarray(np.float32(total / N))


# revision 13
# speedup vs baseline: 1.1754x; 1.1754x over previous
"""Trainium2 Bass kernel for nn_CorrLoss: margin-ranking loss over a Gram matrix.

loss = mean_i relu( max_{j: t_j != t_i} corr[i,j] - min_{j: t_j == t_i} corr[i,j] + 40 )
with corr = feat @ feat.T, feat [4096, 512] f32, targets [4096] int.

Strategy (row-data-parallel over 8 NeuronCores, class-sorted layout):
- Host sorts rows by class. Core c owns sorted rows [512c, 512c+512); its
  column order is the sorted order rotated by -512c, so the core's own rows
  are exactly columns [0, 512) and the stationary matmul operand slices
  directly out of the feature tile.
- The same-class mask is folded into the matmul: a one-hot class block
  scaled by -BIG extends the contraction, so PSUM holds
  scr = corr - BIG*same. Then an = rowmax(scr) and
  ap = rowmin(scr over the positive window) + BIG.
- Class-sorted columns put each core's positives in cols [0, 1024) plus a
  small wrapped tail, so mask matmuls / min-reduces touch chunks {0,1,7}.
- Features are fp8 e4m3 with DoubleRow matmuls (2 k-tiles per instruction at
  0.5 cycles/row); the one-hot mask matmul stays bf16. Measured loss error
  ~6e-4 relative, far under the 2e-2 gate.
- Only the Scalar engine reads PSUM (Vector+Scalar PSUM readers in one
  kernel hard-fault the device). Chunk (n,m) goes to bank 2m + n%2, so
  Scalar drains two banks per 1024-wide copy: the head pair {0,1} and
  chunk 7 are copied f32 (ap needs full precision against the -BIG
  offset), everything else bf16.
- Vector builds the rowwise max incrementally in bf16 (2x tensor_tensor,
  326ns/chunk) as copies land, and min-reduces the f32 window copies.
"""
import sys
from contextlib import ExitStack

import numpy as np

sys.path.insert(0, "/opt/trn_rl_repo")

import concourse.bass as bass  # noqa: E402
from concourse import mybir  # noqa: E402
from concourse.bass_utils import run_bass_kernel_spmd  # noqa: E402

import ml_dtypes  # noqa: E402

BF16 = ml_dtypes.bfloat16
F8 = ml_dtypes.float8_e4m3

N_CORES = 8
N = 4096                # total rows
D = 512                 # feature dim
M = N // N_CORES        # 512 local rows per core
KT = D // 128           # 4 feature k-chunks
MT = M // 128            # 4 row blocks of 128
NCHUNK = 512            # psum chunk width
NT = N // NCHUNK        # 8 col chunks
NQ = 4                  # fT DMA column quarters
QW = N // NQ            # 1024 cols per quarter
MARGIN = 40.0
BIG = 2048.0

KOH = 16                # max distinct classes per core (one-hot depth)
HEADC = 2               # head window = chunks [0, HEADC) -> cols [0, 1024)
TAILW = 256             # tail window = last TAILW cols of chunk NT-1

_CACHE = {}


def _build():
    f32 = mybir.dt.float32
    bf = mybir.dt.bfloat16
    f8 = mybir.dt.float8e4
    op = mybir.AluOpType
    DR = mybir.MatmulPerfMode.DoubleRow
    nc = bass.Bass("TRN2", target_bir_lowering=False, debug=False)

    WCOLS = HEADC * NCHUNK + TAILW
    fTr = nc.declare_dram_parameter("fTr", [NQ, D, QW], f8, isOutput=False)
    oh = nc.declare_dram_parameter("oh", [KOH, M + WCOLS], bf, isOutput=False)
    out2 = nc.declare_dram_parameter("out2", [128, 2 * MT], f32, isOutput=True)

    # scalar drain event index for the copy covering chunk (n, m)
    def drain_idx(n, m):
        if n < 6:
            return (n // 2) * MT + m + 1      # pair copies: events 1..12
        if n == 6:
            return 3 * MT + m + 1             # chunk-6 singles: 13..16
        return 4 * MT + m + 1                 # chunk-7 singles: 17..20

    with ExitStack() as ctx:
        fTs = ctx.enter_context(nc.sbuf_tensor("fTs", [128, KT, N], f8))
        ohs = ctx.enter_context(nc.sbuf_tensor("ohs", [128, M + WCOLS], bf))
        # bf16 copies of chunks 2..6 (chain food)
        sbf = ctx.enter_context(nc.sbuf_tensor("sbf", [128, MT, 5, NCHUNK], bf))
        # f32 copies: head pair {0,1} and chunk 7 (ap precision)
        s32h = ctx.enter_context(nc.sbuf_tensor("s32h", [128, MT, 2 * NCHUNK], f32))
        s32t = ctx.enter_context(nc.sbuf_tensor("s32t", [128, MT, NCHUNK], f32))
        an_run = ctx.enter_context(nc.sbuf_tensor("an_run", [128, MT, NCHUNK], bf))
        ap_acc = ctx.enter_context(nc.sbuf_tensor("ap_acc", [128, MT, 2], f32))
        out_sb = ctx.enter_context(nc.sbuf_tensor("out_sb", [128, 2 * MT], f32))
        warm = ctx.enter_context(nc.sbuf_tensor("warm", [128, 1], f32))
        pa = ctx.enter_context(nc.psum_tensor("pa", [128, 8, NCHUNK], f32))
        oh_sem = ctx.enter_context(nc.semaphore("oh_sem"))
        q_sem = [ctx.enter_context(nc.semaphore(f"q_sem{q}")) for q in range(NQ)]
        mm_sem = ctx.enter_context(nc.semaphore("mm_sem"))
        cp_sem = ctx.enter_context(nc.semaphore("cp_sem"))
        done_sem = ctx.enter_context(nc.semaphore("done_sem"))
        out_sem = ctx.enter_context(nc.semaphore("out_sem"))
        block = ctx.enter_context(nc.Block())

        def ft_piece(eng, q):
            return eng.dma_start(
                fTs[:, :, q * QW:(q + 1) * QW],
                fTr[q].rearrange("(k p) c -> p k c", k=KT),
            )

        @block.sync
        def _(sync):
            ft_piece(sync, 0).then_inc(q_sem[0], 16)
            ft_piece(sync, 3).then_inc(q_sem[3], 16)
            sync.wait_ge(done_sem, 1)
            sync.dma_start(out2[:], out_sb[:]).then_inc(out_sem, 16)
            sync.wait_ge(out_sem, 16)

        @block.gpsimd
        def _(gpsimd):
            gpsimd.dma_start(ohs[0:KOH, :], oh[:, :]).then_inc(oh_sem, 16)
            ft_piece(gpsimd, 2).then_inc(q_sem[2], 16)

        @block.tensor
        def _(tensor):
            for n in range(NT):
                q = (n * NCHUNK) // QW
                tensor.wait_ge(q_sem[q], 16)
                if n == 0:
                    tensor.wait_ge(oh_sem, 16)
                for m in range(MT):
                    b = 2 * m + (n % 2)
                    if n >= 2:
                        tensor.wait_ge(cp_sem, drain_idx(n - 2, m))
                    has_mask = n < HEADC or n == NT - 1
                    for kk in range(KT // 2):
                        last = (kk == KT // 2 - 1) and not has_mask
                        mm = nc.tensor.matmul(
                            pa[:, b, :],
                            fTs[:, 2 * kk:2 * kk + 2, m * 128:(m + 1) * 128],
                            fTs[:, 2 * kk:2 * kk + 2, n * NCHUNK:(n + 1) * NCHUNK],
                            start=(kk == 0), stop=last,
                            perf_mode=DR, skip_group_check=True)
                    if n < HEADC:
                        mm = nc.tensor.matmul(
                            pa[:, b, :],
                            ohs[0:KOH, m * 128:(m + 1) * 128],
                            ohs[0:KOH, M + n * NCHUNK:M + (n + 1) * NCHUNK],
                            start=False, stop=True, skip_group_check=True)
                    elif n == NT - 1:
                        mm = nc.tensor.matmul(
                            pa[:, b, NCHUNK - TAILW:NCHUNK],
                            ohs[0:KOH, m * 128:(m + 1) * 128],
                            ohs[0:KOH, M + HEADC * NCHUNK:M + WCOLS],
                            start=False, stop=True, skip_group_check=True)
                    mm.then_inc(mm_sem, 1)

        @block.scalar
        def _(scalar):
            ft_piece(scalar, 1).then_inc(q_sem[1], 16)
            scalar.wait_ge(oh_sem, 16)
            nc.scalar.copy(warm[0:KOH, :], ohs[0:KOH, 0:1])
            # head pair {0,1} -> f32
            for m in range(MT):
                scalar.wait_ge(mm_sem, 1 * MT + m + 1)
                cp = nc.scalar.copy(
                    s32h[:, m, :].rearrange("p (t c) -> p t c", t=2),
                    pa[:, 2 * m:2 * m + 2, :])
                cp.then_inc(cp_sem, 1)
            # pairs {2,3} and {4,5} -> bf16
            for pr in (1, 2):
                for m in range(MT):
                    scalar.wait_ge(mm_sem, (2 * pr + 1) * MT + m + 1)
                    cp = nc.scalar.copy(
                        sbf[:, m, 2 * pr - 2:2 * pr, :],
                        pa[:, 2 * m:2 * m + 2, :])
                    cp.then_inc(cp_sem, 1)
            # chunk 6 -> bf16, chunk 7 -> f32
            for m in range(MT):
                scalar.wait_ge(mm_sem, 6 * MT + m + 1)
                cp = nc.scalar.copy(sbf[:, m, 4, :], pa[:, 2 * m, :])
                cp.then_inc(cp_sem, 1)
            for m in range(MT):
                scalar.wait_ge(mm_sem, 7 * MT + m + 1)
                cp = nc.scalar.copy(s32t[:, m, :], pa[:, 2 * m + 1, :])
                cp.then_inc(cp_sem, 1)

        @block.vector
        def _(vector):
            for m in range(MT):
                vector.wait_ge(cp_sem, drain_idx(0, m))
                nc.vector.tensor_tensor(
                    an_run[:, m, :], s32h[:, m, 0:NCHUNK],
                    s32h[:, m, NCHUNK:2 * NCHUNK], op=op.max)
                nc.vector.tensor_reduce(
                    ap_acc[:, m, 0:1], s32h[:, m, :],
                    axis=mybir.AxisListType.X, op=op.min)
            for n in range(2, 7):
                for m in range(MT):
                    vector.wait_ge(cp_sem, drain_idx(n, m))
                    nc.vector.tensor_tensor(
                        an_run[:, m, :], an_run[:, m, :],
                        sbf[:, m, n - 2, :], op=op.max)
            for m in range(MT):
                vector.wait_ge(cp_sem, drain_idx(7, m))
                nc.vector.tensor_tensor(
                    an_run[:, m, :], an_run[:, m, :], s32t[:, m, :], op=op.max)
                nc.vector.tensor_reduce(
                    ap_acc[:, m, 1:2], s32t[:, m, NCHUNK - TAILW:NCHUNK],
                    axis=mybir.AxisListType.X, op=op.min)
                nc.vector.tensor_reduce(
                    out_sb[:, MT + m:MT + m + 1], an_run[:, m, :],
                    axis=mybir.AxisListType.X, op=op.max)
                fin = nc.vector.tensor_reduce(
                    out_sb[:, m:m + 1], ap_acc[:, m, :],
                    axis=mybir.AxisListType.X, op=op.min)
                if m == MT - 1:
                    fin.then_inc(done_sem, 1)
    return nc


def _prep_inputs(feat: np.ndarray, targets: np.ndarray):
    """Sort rows by class and build per-core rotated inputs."""
    feat = np.asarray(feat, dtype=np.float32)
    tg = np.asarray(targets).astype(np.int64).ravel()

    order = np.argsort(tg, kind="stable")
    ts = tg[order]                       # sorted targets
    fT_s = np.ascontiguousarray(feat[order].T)   # [512, 4096] f32, sorted cols

    WCOLS = HEADC * NCHUNK + TAILW
    in_maps = []
    for c in range(N_CORES):
        base = M * c
        tcol = np.roll(ts, -base)
        fTp = np.roll(fT_s, -base, axis=1).astype(F8)
        fTr = np.ascontiguousarray(
            fTp.reshape(D, NQ, QW).transpose(1, 0, 2))   # [NQ, D, QW]

        c0 = int(ts[base])
        c1 = int(ts[base + M - 1])
        span = c1 - c0 + 1
        p0 = int(np.searchsorted(ts, c0, "left"))
        p1 = int(np.searchsorted(ts, c1, "right"))
        head_w = p1 - base
        tail_w = base - p0
        assert span <= KOH, f"class span {span} > {KOH}"
        assert head_w <= HEADC * NCHUNK, f"head window {head_w}"
        assert tail_w <= TAILW, f"tail window {tail_w}"

        ohx = np.zeros((KOH, M + WCOLS), dtype=np.float32)
        tloc = tcol[:M]
        ohx[tloc - c0, np.arange(M)] = -BIG
        hidx = tcol[:HEADC * NCHUNK] - c0
        hsel = (hidx >= 0) & (hidx < span)
        ohx[hidx[hsel], M + np.nonzero(hsel)[0]] = 1.0
        tidx = tcol[N - TAILW:] - c0
        tsel = (tidx >= 0) & (tidx < span)
        ohx[tidx[tsel], M + HEADC * NCHUNK + np.nonzero(tsel)[0]] = 1.0

        in_maps.append({"fTr": fTr, "oh": ohx.astype(BF16)})
    return in_maps


def kernel(feat: np.ndarray, targets: np.ndarray) -> np.ndarray:
    in_maps = _prep_inputs(feat, targets)

    if "nc" not in _CACHE:
        _CACHE["nc"] = _build()
    nc = _CACHE["nc"]

    res = run_bass_kernel_spmd(nc, in_maps, list(range(N_CORES)))
    total = 0.0
    for c in range(N_CORES):
        o = res.results[c]["out2"].astype(np.float64)
        ap = o[:, :MT] + BIG
        an = o[:, MT:]
        total += np.maximum(an - ap + MARGIN, 0.0).sum()
    return np.asarray(np.float32(total / N))
